# revision 15
# baseline (speedup 1.0000x reference)
"""Trainium2 Bass kernel for nn_Decoder_31387620999286.

4-layer GPT decoder: B=2, T=1024, D=512, H=8, HS=64, V=32000.
Sharding: 8 cores = 2 batch groups x 4 TP ranks.
  core c: batch g=c//4, rank r=c%4
  - attention: heads (2r, 2r+1) of batch g
  - FFN: hidden slice [512r:512(r+1)]
  - proj: rows [128r:128(r+1)]
  - lm_head: vocab slice [8000r:8000(r+1)] for batch g
Activations in "DxT" layout on device: [D partitions, T free].
LayerNorm gains/biases folded into weights host-side; LN stats via PE-ones.
Attention: fwd softmax [t, t'] for attn_maps; transposed scores recomputed
for the AV contraction; AV normalized by broadcast reciprocal denominators.
Two AllReduces per layer (proj partial, FFN partial) over 4-rank groups.
"""

import numpy as np

V, D, H, L, T = 32000, 512, 8, 4, 1024
HS = 64
EPS = 1e-5
P = 128
NC = 8
TPR = 4        # ranks per batch group
HL = 2         # heads per core
DC = D // P    # 4 D-chunks
TC = T // P    # 8 t-chunks
FL = 2048 // TPR   # 512 ffn hidden per core
VS = V // TPR      # 8000 vocab cols per core
VW = 500           # vocab chunk width
VC = VS // VW      # 16 vocab chunks
NEG = -1.0e30

_CACHE = {}


def _build():
    import concourse.bass as bass
    import concourse.mybir as mybir
    import concourse.tile as tile
    from concourse import bacc

    f32 = mybir.dt.float32
    f32r = mybir.dt.float32r

    nc = bacc.Bacc("TRN2", target_bir_lowering=False, num_devices=NC)

    # ---- I/O -------------------------------------------------------------
    x0_d = nc.dram_tensor("x0", [D, T], f32r, kind="ExternalInput")
    wqk_d = nc.dram_tensor("wqk", [L, D, 2 * P], f32r, kind="ExternalInput")
    wv_d = nc.dram_tensor("wv", [L, D, P], f32r, kind="ExternalInput")
    bq_d = nc.dram_tensor("bq", [P, L], f32, kind="ExternalInput")
    bk_d = nc.dram_tensor("bk", [P, L], f32, kind="ExternalInput")
    bv_d = nc.dram_tensor("bv", [1, L * P], f32r, kind="ExternalInput")
    wproj_d = nc.dram_tensor("wproj", [L, P, D], f32r, kind="ExternalInput")
    bp4_d = nc.dram_tensor("bp4", [P, L, DC], f32, kind="ExternalInput")
    w1_d = nc.dram_tensor("w1", [L, D, FL], f32r, kind="ExternalInput")
    b1e_d = nc.dram_tensor("b1e", [P, L, FL // P], f32, kind="ExternalInput")
    w2_d = nc.dram_tensor("w2", [L, FL, D], f32r, kind="ExternalInput")
    b2q_d = nc.dram_tensor("b2q", [P, L, DC], f32, kind="ExternalInput")
    lmw_d = nc.dram_tensor("lmw", [D, VS], f32r, kind="ExternalInput")
    lmb_d = nc.dram_tensor("lmb", [1, VS], f32r, kind="ExternalInput")

    attn_d = nc.dram_tensor("attn", [L, HL, T, T], f32, kind="ExternalOutput")
    logits_d = nc.dram_tensor("logits", [T, VS], f32, kind="ExternalOutput")
    sumexp_d = nc.dram_tensor("sumexp", [P, TC], f32, kind="ExternalOutput")

    HALF = [slice(0, 512), slice(512, 1024)]

    with tile.TileContext(nc) as tc:
        import contextlib
        ctx = contextlib.ExitStack()
        with ctx:
            singles = ctx.enter_context(tc.tile_pool(name="singles", bufs=1))
            wpool = ctx.enter_context(tc.tile_pool(name="wpool", bufs=1))
            xpool = ctx.enter_context(tc.tile_pool(name="xpool", bufs=1))
            stat = ctx.enter_context(tc.tile_pool(name="stat", bufs=1))
            attnp = ctx.enter_context(tc.tile_pool(name="attnp", bufs=1))
            psp = ctx.enter_context(tc.tile_pool(name="psp", bufs=1, space="PSUM"))
            drp = ctx.enter_context(tc.tile_pool(name="drp", bufs=2, space="DRAM"))

            def pb(name):
                return psp.tile([P, 512], f32, tag="pb1", bufs=4, name=name)

            # ---- constants ---------------------------------------------
            onesf = singles.tile([P, P], f32)
            nc.gpsimd.memset(onesf[:], 1.0)
            ones1 = singles.tile([1, P], f32r)        # K=1 row of ones
            nc.vector.tensor_copy(ones1[:], onesf[0:1, :])
            onesD = singles.tile([P, 1], f32r)        # ones column, K=128
            nc.vector.tensor_copy(onesD[:], onesf[:, 0:1])
            sel2f = singles.tile([2, P], f32)
            nc.gpsimd.memset(sel2f[:], 1.0)
            # row 0: ones in cols [0,64); row 1: ones in cols [64,128)
            nc.gpsimd.affine_select(
                out=sel2f[:], in_=sel2f[:], compare_op=mybir.AluOpType.is_ge,
                fill=0.0, base=63, channel_multiplier=1000, pattern=[[-1, P]])
            nc.gpsimd.affine_select(
                out=sel2f[:], in_=sel2f[:], compare_op=mybir.AluOpType.is_ge,
                fill=0.0, base=936, channel_multiplier=-1000, pattern=[[1, P]])
            sel2 = singles.tile([2, P], f32r)         # head-selector for bcast
            nc.vector.tensor_copy(sel2[:], sel2f[:])
            ident = singles.tile([P, P], f32)
            nc.gpsimd.memset(ident[:], 0.0)
            nc.gpsimd.affine_select(
                out=ident[:], in_=ident[:], compare_op=mybir.AluOpType.not_equal,
                fill=1.0, base=0, pattern=[[-1, P]], channel_multiplier=1,
            )
            eps_s = singles.tile([1, 1], f32)
            nc.vector.memset(eps_s[:], EPS)

            # biases resident
            bq_s = singles.tile([P, L], f32)
            bk_s = singles.tile([P, L], f32)
            nc.sync.dma_start(bq_s[:], bq_d[:])
            nc.sync.dma_start(bk_s[:], bk_d[:])
            bv_s = singles.tile([1, L, P], f32r)
            nc.sync.dma_start(bv_s[:], bv_d[:].rearrange("o (l p) -> o l p", p=P))
            bp4_s = singles.tile([P, L, DC], f32)
            nc.sync.dma_start(bp4_s[:], bp4_d[:])
            b1e_s = singles.tile([P, L, FL // P], f32)
            nc.sync.dma_start(b1e_s[:], b1e_d[:])
            b2q_s = singles.tile([P, L, DC], f32)
            nc.sync.dma_start(b2q_s[:], b2q_d[:])

            # ---- layernorm (stats via PE-ones, returns f32r xn) --------
            def layer_norm(x_s):
                m = stat.tile([1, T], f32, tag="m", name="m")
                msq = stat.tile([1, T], f32, tag="msq", name="msq")
                for n in range(2):
                    sl = HALF[n]
                    s1_ps = pb("s1_ps")
                    for kc in range(DC):
                        nc.tensor.matmul(
                            s1_ps[0:1, :], onesD[:], x_s[:, kc, sl],
                            start=(kc == 0), stop=(kc == DC - 1),
                        )
                    nc.scalar.mul(m[:, sl], s1_ps[0:1, :], 1.0 / D)
                    s2_ps = pb("s2_ps")
                    for kc in range(DC):
                        xsq = stat.tile([P, 512], f32r, tag="xsq", bufs=1,
                                        name="xsq")
                        nc.vector.tensor_mul(
                            xsq[:],
                            x_s[:, kc, sl].bitcast(f32),
                            x_s[:, kc, sl].bitcast(f32),
                        )
                        nc.tensor.matmul(
                            s2_ps[0:1, :], onesD[:], xsq[:],
                            start=(kc == 0), stop=(kc == DC - 1),
                        )
                    nc.scalar.mul(msq[:, sl], s2_ps[0:1, :], 1.0 / D)
                mm = stat.tile([1, T], f32, tag="mm", name="mm")
                nc.vector.tensor_mul(mm[:], m[:], m[:])
                nc.vector.tensor_sub(msq[:], msq[:], mm[:])   # msq := var
                sd = stat.tile([1, T], f32, tag="mm", name="sd")
                nc.scalar.activation(
                    sd[:], msq[:], mybir.ActivationFunctionType.Sqrt,
                    bias=eps_s[:],
                )
                r_r = stat.tile([1, T], f32r, tag="r_r", name="r_r")
                with nc.allow_low_precision(reason="fp32r is fp32-width"):
                    nc.vector.reciprocal(r_r[:], sd[:])
                rm_r = stat.tile([1, T], f32r, tag="rm_r", name="rm_r")
                nc.vector.tensor_mul(rm_r[:], r_r[:].bitcast(f32), m[:])
                # broadcast r, rm across partitions via PE
                rB = stat.tile([P, T], f32, tag="rB_s", name="rB")
                rmB = stat.tile([P, T], f32, tag="rmB_s", name="rmB")
                for n in range(2):
                    sl = HALF[n]
                    rB_ps = pb("rB_ps")
                    nc.tensor.matmul(rB_ps[:], ones1[:], r_r[:, sl],
                                     start=True, stop=True)
                    nc.scalar.copy(rB[:, sl], rB_ps[:])
                    rmB_ps = pb("rmB_ps")
                    nc.tensor.matmul(rmB_ps[:], ones1[:], rm_r[:, sl],
                                     start=True, stop=True)
                    nc.scalar.copy(rmB[:, sl], rmB_ps[:])
                xn = xpool.tile([P, DC, T], f32r, tag="xn", bufs=2, name="xn")
                for kc in range(DC):
                    nc.vector.tensor_mul(
                        xn[:, kc, :], x_s[:, kc, :].bitcast(f32), rB[:])
                    nc.vector.tensor_sub(xn[:, kc, :], xn[:, kc, :], rmB[:])
                return xn

            # ---- AllReduce over the 4-rank batch group -----------------
            def allreduce(src_sb):
                b_in = drp.tile([D, T], f32, tag="ar_in", name="b_in")
                b_out = drp.tile([D, T], f32, tag="ar_out", name="b_out")
                nc.sync.dma_start(
                    b_in[:].rearrange("(c p) t -> p c t", p=P), src_sb[:])
                nc.gpsimd.collective_compute(
                    "AllReduce",
                    mybir.AluOpType.add,
                    replica_groups=[[0, 1, 2, 3], [4, 5, 6, 7]],
                    ins=[b_in.opt()],
                    outs=[b_out.opt()],
                )
                dst = xpool.tile([P, DC, T], f32, tag="xn", bufs=2, name="dst")
                nc.sync.dma_start(
                    dst[:], b_out[:].rearrange("(c p) t -> p c t", p=P))
                return dst

            # ---- residual stream ---------------------------------------
            x_s = xpool.tile([P, DC, T], f32r, tag="x", name="x_s")
            nc.sync.dma_start(x_s[:], x0_d[:].rearrange("(c p) t -> p c t", p=P))

            for l in range(L):
                # -- weights for this layer
                wqk_s = wpool.tile([P, DC, 2 * P], f32r, tag="wqk", bufs=2,
                                   name="wqk_s")
                nc.sync.dma_start(
                    wqk_s[:], wqk_d[l].rearrange("(c p) m -> p c m", p=P))
                wv_s = wpool.tile([P, DC, P], f32r, tag="wv", bufs=2,
                                  name="wv_s")
                nc.sync.dma_start(
                    wv_s[:], wv_d[l].rearrange("(c p) m -> p c m", p=P))
                wproj_s = wpool.tile([P, D], f32r, tag="wproj", bufs=2,
                                     name="wproj_s")
                nc.sync.dma_start(wproj_s[:], wproj_d[l])
                w1_s = wpool.tile([P, DC, FL], f32r, tag="w1", name="w1_s")
                nc.sync.dma_start(
                    w1_s[:], w1_d[l].rearrange("(c p) m -> p c m", p=P))
                w2_s = wpool.tile([P, FL // P, D], f32r, tag="w2", name="w2_s")
                nc.sync.dma_start(
                    w2_s[:], w2_d[l].rearrange("(c p) m -> p c m", p=P))

                xn = layer_norm(x_s)

                # -- q, k in DxT (head dims on partitions)
                q_s = attnp.tile([P, T], f32r, tag="q_s", name="q_s")
                k_s = attnp.tile([P, T], f32r, tag="k_s", name="k_s")
                for n in range(2):
                    sl = HALF[n]
                    q_ps = pb("q_ps")
                    for kc in range(DC):
                        nc.tensor.matmul(
                            q_ps[:], wqk_s[:, kc, 0:P], xn[:, kc, sl],
                            start=(kc == 0), stop=(kc == DC - 1))
                    nc.scalar.add(q_s[:, sl], q_ps[:], bq_s[:, l:l + 1])
                    k_ps = pb("k_ps")
                    for kc in range(DC):
                        nc.tensor.matmul(
                            k_ps[:], wqk_s[:, kc, P:2 * P], xn[:, kc, sl],
                            start=(kc == 0), stop=(kc == DC - 1))
                    nc.scalar.add(k_s[:, sl], k_ps[:], bk_s[:, l:l + 1])

                # -- vT: [t-prime partitions, head dims free]
                vT_s = attnp.tile([P, TC, P], f32r, tag="vT", name="vT_s")
                for i in range(TC):
                    v_ps = pb("v_ps")
                    for kc in range(DC):
                        nc.tensor.matmul(
                            v_ps[:, 0:P], xn[:, kc, P * i:P * (i + 1)],
                            wv_s[:, kc, :],
                            start=(kc == 0), stop=False)
                    nc.tensor.matmul(
                        v_ps[:, 0:P], ones1[:], bv_s[:, l, :],
                        start=False, stop=True)
                    nc.scalar.copy(vT_s[:, i, :], v_ps[:, 0:P])

                # -- forward softmax (per q-chunk, per head) + attn_maps out
                rd2 = stat.tile([2, T], f32r, tag="rd2", name="rd2")
                for i in range(TC):
                    rden2 = stat.tile([P, 2], f32, tag="rden2", bufs=2,
                                      name="rden2")
                    for h in range(HL):
                        hsl = slice(HS * h, HS * (h + 1))
                        sm = attnp.tile([P, T], f32, tag="smwei", bufs=3,
                                        name="sm")
                        for n in range(2):
                            sl = HALF[n]
                            s_ps = pb("s_ps")
                            nc.tensor.matmul(
                                s_ps[:],
                                q_s[hsl, P * i:P * (i + 1)],
                                k_s[hsl, sl],
                                start=True, stop=True)
                            nc.vector.tensor_copy(sm[:, sl], s_ps[:])
                        nc.gpsimd.affine_select(
                            out=sm[:], in_=sm[:],
                            compare_op=mybir.AluOpType.is_ge, fill=NEG,
                            base=P * i, pattern=[[-1, T]], channel_multiplier=1)
                        wei = attnp.tile([P, T], f32, tag="smwei", bufs=3,
                                         name="wei")
                        den = stat.tile([P, 1], f32, tag="den", bufs=2,
                                        name="den")
                        nc.scalar.activation(
                            wei[:], sm[:], mybir.ActivationFunctionType.Exp,
                            accum_out=den[:])
                        nc.vector.reciprocal(rden2[:, h:h + 1], den[:])
                        nc.vector.tensor_scalar_mul(
                            wei[:], wei[:], rden2[:, h:h + 1])
                        nc.sync.dma_start(
                            attn_d[l, h, P * i:P * (i + 1), :], wei[:])
                    # transpose both heads' 1/den into rd2 cols [Pi, Pi+128)
                    rd_ps = pb("rd_ps")
                    nc.tensor.transpose(rd_ps[0:2, 0:P], rden2[:], ident[:])
                    with nc.allow_low_precision(reason="fp32r copy"):
                        nc.vector.tensor_copy(
                            rd2[:, P * i:P * (i + 1)], rd_ps[0:2, 0:P])

                # -- transposed scores -> eT -> av (accumulate over t-chunks)
                av_ps = [[psp.tile([HS, 512], f32, tag=f"av{n}{h}",
                                   name=f"av{n}{h}")
                          for h in range(HL)] for n in range(2)]
                for j in range(TC):
                    for h in range(HL):
                        hsl = slice(HS * h, HS * (h + 1))
                        eT = attnp.tile([P, T], f32r, tag="eT", bufs=2,
                                        name="eT")
                        for n in range(2):
                            sl = HALF[n]
                            sT_ps = pb("sT_ps")
                            nc.tensor.matmul(
                                sT_ps[:],
                                k_s[hsl, P * j:P * (j + 1)],
                                q_s[hsl, sl],
                                start=True, stop=True)
                            nc.scalar.activation(
                                eT[:, sl], sT_ps[:],
                                mybir.ActivationFunctionType.Exp)
                        nc.gpsimd.affine_select(
                            out=eT[:], in_=eT[:],
                            compare_op=mybir.AluOpType.is_ge, fill=0.0,
                            base=-P * j, pattern=[[1, T]], channel_multiplier=-1)
                        for n in range(2):
                            sl = HALF[n]
                            nc.tensor.matmul(
                                av_ps[n][h][:],
                                vT_s[:, j, hsl],
                                eT[:, sl],
                                start=(j == 0), stop=(j == TC - 1))
                # normalize av by broadcast 1/den over t (free axis)
                av_s = attnp.tile([P, T], f32r, tag="smwei", bufs=3,
                                  name="av_s")
                for n in range(2):
                    sl = HALF[n]
                    bc_ps = pb("bc_ps")
                    nc.tensor.matmul(bc_ps[:], sel2[:], rd2[:, sl],
                                     start=True, stop=True)
                    for h in range(HL):
                        nc.scalar.copy(
                            av_s[HS * h:HS * (h + 1), sl], av_ps[n][h][:])
                    nc.vector.tensor_mul(av_s[:, sl], av_s[:, sl], bc_ps[:])

                # -- proj partial + AllReduce + residual
                arin = xpool.tile([P, DC, T], f32, tag="arin", name="arin")
                for mc in range(DC):
                    for n in range(2):
                        sl = HALF[n]
                        pp_ps = pb("pp_ps")
                        nc.tensor.matmul(
                            pp_ps[:], wproj_s[:, P * mc:P * (mc + 1)],
                            av_s[:, sl], start=True, stop=True)
                        nc.vector.tensor_scalar_add(
                            arin[:, mc, sl], pp_ps[:], bp4_s[:, l, mc:mc + 1])
                ar1 = allreduce(arin)
                x2 = xpool.tile([P, DC, T], f32r, tag="x2", name="x2")
                for kc in range(DC):
                    nc.vector.tensor_add(
                        x2[:, kc, :], x_s[:, kc, :].bitcast(f32), ar1[:, kc, :])

                xn2 = layer_norm(x2)

                # -- FFN
                h_s = xpool.tile([P, FL // P, T], f32r, tag="h_s", name="h_s")
                for hc in range(FL // P):
                    for n in range(2):
                        sl = HALF[n]
                        h_ps = pb("h_ps")
                        for kc in range(DC):
                            nc.tensor.matmul(
                                h_ps[:], w1_s[:, kc, P * hc:P * (hc + 1)],
                                xn2[:, kc, sl],
                                start=(kc == 0), stop=(kc == DC - 1))
                        nc.scalar.activation(
                            h_s[:, hc, sl], h_ps[:],
                            mybir.ActivationFunctionType.Relu,
                            bias=b1e_s[:, l, hc:hc + 1])
                arin2 = xpool.tile([P, DC, T], f32, tag="arin", name="arin2")
                for mc in range(DC):
                    for n in range(2):
                        sl = HALF[n]
                        f_ps = pb("f_ps")
                        for hc in range(FL // P):
                            nc.tensor.matmul(
                                f_ps[:], w2_s[:, hc, P * mc:P * (mc + 1)],
                                h_s[:, hc, sl],
                                start=(hc == 0), stop=(hc == FL // P - 1))
                        nc.vector.tensor_scalar_add(
                            arin2[:, mc, sl], f_ps[:], b2q_s[:, l, mc:mc + 1])
                ar2 = allreduce(arin2)
                x3 = xpool.tile([P, DC, T], f32r, tag="x", name="x3")
                for kc in range(DC):
                    nc.vector.tensor_add(
                        x3[:, kc, :], x2[:, kc, :].bitcast(f32), ar2[:, kc, :])
                x_s = x3

            # ---- final LN + lm_head -----------------------------------
            xfn = layer_norm(x_s)
            sx_all = stat.tile([P, TC], f32, tag="sx_all", name="sx_all")
            for j in range(VC):
                lmw_s = wpool.tile([P, DC, VW], f32r, tag="w1", name="lmw_s")
                nc.sync.dma_start(
                    lmw_s[:],
                    lmw_d[:].rearrange("(c p) v -> p c v", p=P)[
                        :, :, VW * j:VW * (j + 1)])
                lmb_c = stat.tile([1, VW], f32r, tag="lmb_c", bufs=2,
                                  name="lmb_c")
                nc.sync.dma_start(lmb_c[:], lmb_d[:, VW * j:VW * (j + 1)])
                for i in range(TC):
                    lg_ps = pb("lg_ps")[:, 0:VW]
                    for kc in range(DC):
                        nc.tensor.matmul(
                            lg_ps[:], xfn[:, kc, P * i:P * (i + 1)],
                            lmw_s[:, kc, :],
                            start=(kc == 0), stop=False)
                    nc.tensor.matmul(
                        lg_ps[:], ones1[:], lmb_c[:],
                        start=False, stop=True)
                    lg_sb = attnp.tile([P, VW], f32, tag="lg_sb", bufs=2,
                                       name="lg_sb")
                    nc.vector.tensor_copy(lg_sb[:], lg_ps[:])
                    nc.sync.dma_start(
                        logits_d[P * i:P * (i + 1), VW * j:VW * (j + 1)],
                        lg_sb[:])
                    sc = attnp.tile([P, VW], f32, tag="lg_sb", bufs=2,
                                    name="sc")
                    sx_col = stat.tile([P, 1], f32, tag="sx_col", bufs=2,
                                       name="sx_col")
                    nc.scalar.activation(
                        sc[:], lg_ps[:], mybir.ActivationFunctionType.Exp,
                        accum_out=sx_col[:])
                    if j == 0:
                        nc.vector.tensor_copy(sx_all[:, i:i + 1], sx_col[:])
                    else:
                        nc.vector.tensor_add(
                            sx_all[:, i:i + 1], sx_all[:, i:i + 1], sx_col[:])
            nc.sync.dma_start(sumexp_d[:], sx_all[:])

    nc.finalize()
    return nc


def _prep_inputs(inputs):
    """Host-side: fold LN params into weights, slice per core, embed tokens."""
    f = np.float32
    idx = np.asarray(inputs["idx"])
    tok_emb = np.asarray(inputs["tok_emb"], f)
    pos_emb = np.asarray(inputs["pos_emb"], f)
    wq = np.asarray(inputs["wq"], f)
    wk = np.asarray(inputs["wk"], f)
    wv = np.asarray(inputs["wv"], f)
    wproj = np.asarray(inputs["wproj"], f)
    bproj = np.asarray(inputs["bproj"], f)
    ln1_g = np.asarray(inputs["ln1_g"], f)
    ln1_b = np.asarray(inputs["ln1_b"], f)
    ln2_g = np.asarray(inputs["ln2_g"], f)
    ln2_b = np.asarray(inputs["ln2_b"], f)
    w1 = np.asarray(inputs["w1"], f)
    b1 = np.asarray(inputs["b1"], f)
    w2 = np.asarray(inputs["w2"], f)
    b2 = np.asarray(inputs["b2"], f)
    lnf_g = np.asarray(inputs["lnf_g"], f)
    lnf_b = np.asarray(inputs["lnf_b"], f)
    lm_w = np.asarray(inputs["lm_w"], f)
    lm_b = np.asarray(inputs["lm_b"], f)

    # scale q by 1/sqrt(HS) and fold ln1_g into wq/wk/wv rows
    wq_g = wq * ln1_g[:, None, :, None] * np.float32(HS ** -0.5)
    wk_g = wk * ln1_g[:, None, :, None]
    wv_g = wv * ln1_g[:, None, :, None]
    # biases from folded ln1_b
    bq_full = np.einsum("ld,lhds->lhs", ln1_b, wq_g).astype(f)  # [L,H,HS]
    bk_full = np.einsum("ld,lhds->lhs", ln1_b, wk_g).astype(f)
    bv_full = np.einsum("ld,lhds->lhs", ln1_b, wv_g).astype(f)
    w1_g = w1 * ln2_g[:, :, None]
    b1e_full = b1 + np.einsum("ld,ldm->lm", ln2_b, w1_g).astype(f)  # [L,2048]
    lmw_g = lm_w * lnf_g[:, None]
    lmb_full = lm_b + (lnf_b @ lmw_g).astype(f)  # [V]

    in_maps = []
    for c in range(NC):
        g, r = divmod(c, TPR)
        hh = [2 * r, 2 * r + 1]
        x0 = (tok_emb[idx[g]] + pos_emb[:T]).astype(f).T.copy()  # [D,T]
        wqk = np.concatenate(
            [wq_g[:, hh[0]], wq_g[:, hh[1]], wk_g[:, hh[0]], wk_g[:, hh[1]]],
            axis=2).astype(f)  # [L,D,256]
        wv_loc = np.concatenate(
            [wv_g[:, hh[0]], wv_g[:, hh[1]]], axis=2).astype(f)  # [L,D,128]
        bq = np.concatenate([bq_full[:, hh[0]], bq_full[:, hh[1]]], 1).T.copy()
        bk = np.concatenate([bk_full[:, hh[0]], bk_full[:, hh[1]]], 1).T.copy()
        bv = np.concatenate(
            [bv_full[:, hh[0]], bv_full[:, hh[1]]], 1).reshape(1, L * P).copy()
        wproj_loc = wproj[:, P * r:P * (r + 1), :].copy()  # [L,128,D]
        bp4 = (bproj / TPR).reshape(L, DC, P).transpose(2, 0, 1).copy()
        w1_loc = w1_g[:, :, FL * r:FL * (r + 1)].copy()
        b1e = b1e_full[:, FL * r:FL * (r + 1)].reshape(
            L, FL // P, P).transpose(2, 0, 1).copy()
        w2_loc = w2[:, FL * r:FL * (r + 1), :].copy()
        b2q = (b2 / TPR).reshape(L, DC, P).transpose(2, 0, 1).copy()
        lmw_loc = lmw_g[:, VS * r:VS * (r + 1)].astype(f).copy()
        lmb_loc = lmb_full[VS * r:VS * (r + 1)].reshape(1, VS).astype(f).copy()
        in_maps.append({
            "x0": np.ascontiguousarray(x0),
            "wqk": np.ascontiguousarray(wqk),
            "wv": np.ascontiguousarray(wv_loc),
            "bq": np.ascontiguousarray(bq.astype(f)),
            "bk": np.ascontiguousarray(bk.astype(f)),
            "bv": np.ascontiguousarray(bv.astype(f)),
            "wproj": np.ascontiguousarray(wproj_loc),
            "bp4": np.ascontiguousarray(bp4.astype(f)),
            "w1": np.ascontiguousarray(w1_loc.astype(f)),
            "b1e": np.ascontiguousarray(b1e.astype(f)),
            "w2": np.ascontiguousarray(w2_loc.astype(f)),
            "b2q": np.ascontiguousarray(b2q.astype(f)),
            "lmw": np.ascontiguousarray(lmw_loc),
            "lmb": np.ascontiguousarray(lmb_loc),
        })
    return in_maps


def kernel(**inputs):
    from concourse.bass_utils import run_bass_kernel_spmd

    if "nc" not in _CACHE:
        _CACHE["nc"] = _build()
    nc = _CACHE["nc"]

    in_maps = _prep_inputs(inputs)
    res = run_bass_kernel_spmd(nc, in_maps, core_ids=list(range(NC)))
    results = res.results

    f = np.float32
    logits = np.empty((2, T, V), f)
    attn = np.empty((L, H, 2, T, T), f)
    sumexp = np.empty((2, TPR, T), f)
    for c in range(NC):
        g, r = divmod(c, TPR)
        logits[g, :, VS * r:VS * (r + 1)] = results[c]["logits"]
        attn[:, 2 * r:2 * r + 2, g] = results[c]["attn"]
        sumexp[g, r] = results[c]["sumexp"].T.reshape(T)

    # loss = -mean(log_softmax(logits)[targets])
    targets = np.asarray(inputs["targets"]).astype(np.int64)
    lse = np.log(sumexp.sum(axis=1))  # [2, T]
    tgt_logit = np.take_along_axis(
        logits, targets[:, :, None], axis=2)[:, :, 0]
    loss = np.asarray((lse - tgt_logit).mean(), f)

    return logits, loss, attn
